# revision 19
# baseline (speedup 1.0000x reference)
"""Multi-head self-attention (N=2, S=4096, D=1024, H=16) on 8 trn2 cores.

Sharding: data-parallel over batch (2) x tensor-parallel over head groups
(4 heads per core). Core c handles batch b=c//4, head group g=c%4
(heads 4g..4g+3, i.e. output columns 256g..256g+256). No cross-device
comms: heads are independent.

v3 design (vs the v2 "kTz zero-padded K=128" baseline):
  - ST score matmuls run at their true K=64 as row-tiled pairs: the jc
    and jc+1 key chunks of the SAME head execute concurrently in array
    rows 0-63 / 64-127 (tile_position (0,0)/(64,0)), halving ST time.
    This needs k^T and q^T present in BOTH partition halves: the
    projection writes the natural pair layout (even head lo / odd head
    hi) and a SBUF->SBUF DMA fills a swapped copy.
  - exp split across two engines: most chunks on ScalarE (ACTIVATE Exp,
    the accuracy reference), a tunable subset on VectorE via a
    Schraudolph bitcast exp: e = bitcast_bf16(int16(A*st + B)) in one
    tensor_scalar (mult, add) op. Per-element error ~3.3% max; applied
    to a fraction phi of key chunks the output rel-err grows as
    3.3%*sqrt(phi).
  - PV per head at M=65 (64 v columns + ones column accumulating the
    softmax denominator); single ot accumulator [65,1024].
  - PSUM: stp 3x[128,1024] (6 banks, also serves projection / epilogue
    scratch) + otp 1x[65,1024] (2 banks) = 8.
  - epilogue in bf16: PE-transpose [65,128] blocks, batched reciprocal
    over the 8 denominator columns, per-block scalar multiply, one DMA
    per (head, ic).
"""

import numpy as np

import concourse.bacc as bacc
import concourse.tile as tile
import concourse.mybir as mybir
from concourse.bass_utils import run_bass_kernel_spmd
from concourse.masks import make_identity

F32 = mybir.dt.float32
BF16 = mybir.dt.bfloat16
FP16 = mybir.dt.float16
I16 = mybir.dt.int16
Exp = mybir.ActivationFunctionType.Exp
MULT = mybir.AluOpType.mult
ADD = mybir.AluOpType.add

N, S, D = 2, 4096, 1024
H = 16
HD = D // H                      # 64
N_CORES = 8
HPC = H // (N_CORES // N)        # heads per core = 4
MPC = HPC * HD                   # out columns per core = 256
SCALE = 1.0 / np.sqrt(HD)        # post-matmul softmax scale

IC = 1024                        # i-chunk (query cols per exp instruction)
N_IC = S // IC                   # 4
N_JC = S // 128                  # 32 key chunks
N_P = N_JC // 2                  # 16 jc-pairs per (head, ic)
N_SC = S // 512                  # 8 projection s-chunks
N_DT = D // 128                  # 8 contraction tiles
VW = HD + 1                      # vaug stride per head (64 v + 1 ones)

# Schraudolph bitcast exp: e ~= bitcast_bf16(int16(EXP_A*st + EXP_B)).
EXP_A = float(SCALE * np.log2(np.e) * 128.0)
EXP_B = 16250.75
# every exp gets a global index g; it runs on the DVE when
# (g * DVE_NUM) % DVE_DEN < DVE_NUM evenly interleaves phi = NUM/DEN.
DVE_NUM, DVE_DEN = 1, 2          # phi = 0.5, strict alternation


def build_attention_kernel():
    nc = bacc.Bacc(
        "TRN2", target_bir_lowering=False, debug=False,
        enable_asserts=False, num_devices=N_CORES,
    )
    xT = nc.dram_tensor("xT", [D, S], FP16, kind="ExternalInput").ap()
    wqT = nc.dram_tensor("wqT", [D, MPC], FP16, kind="ExternalInput").ap()
    wkT = nc.dram_tensor("wkT", [D, MPC], FP16, kind="ExternalInput").ap()
    wvT = nc.dram_tensor("wvT", [D, MPC], FP16, kind="ExternalInput").ap()
    out = nc.dram_tensor("out", [S, MPC], F32, kind="ExternalOutput").ap()

    with tile.TileContext(nc) as tc:
        _emit(tc, xT, wqT, wkT, wvT, out)
    nc.compile()
    return nc


def _emit(tc, xT, wqT, wkT, wvT, out):
    nc = tc.nc
    with (
        tc.tile_pool(name="persist", bufs=1) as persist,
        # PSUM: stp 3x4KB slots (6 banks; ST tiles + projection ps +
        # epilogue transposes) + otp 1x[65,1024] (2 banks) = 8 banks.
        tc.tile_pool(name="stp", bufs=3, space="PSUM") as stp,
        tc.tile_pool(name="otp", bufs=1, space="PSUM") as otp,
        tc.tile_pool(name="xload", bufs=2) as xload,
        tc.tile_pool(name="esb", bufs=6) as esb,
        tc.tile_pool(name="episb", bufs=2) as episb,
        tc.tile_pool(name="osb", bufs=4) as osb,
    ):
        w_sb = {}
        for name, w in (("q", wqT), ("k", wkT), ("v", wvT)):
            t = persist.tile([128, N_DT, MPC], FP16, tag=f"w{name}")
            for dt in range(N_DT):
                nc.sync.dma_start(out=t[:, dt, :], in_=w[dt * 128:(dt + 1) * 128, :])
            w_sb[name] = t
        # pair layout: plane mt, partitions 0-63 = head 2mt, 64-127 =
        # head 2mt+1. swap layout: partition halves exchanged (via DMA).
        qTp = persist.tile([128, 2, S], FP16, tag="qTp")
        qTs = persist.tile([128, 2, S], FP16, tag="qTs")
        kTp = persist.tile([128, 2, S], FP16, tag="kTp")
        kTs = persist.tile([128, 2, S], FP16, tag="kTs")
        vaug = persist.tile([128, N_JC, HPC * VW], BF16, tag="vaug")
        ident = persist.tile([128, 128], F32, tag="ident")
        make_identity(nc, ident)
        identb = persist.tile([128, 128], BF16, tag="identb")
        nc.vector.tensor_copy(identb[:], ident[:])

        ones_src = persist.tile([128, HPC], F32, tag="ones")
        nc.vector.memset(ones_src, 1.0)
        for jc in range(N_JC):          # vaug ones columns
            nc.vector.tensor_copy(
                vaug[:, jc, :].rearrange(
                    "p (h c) -> p h c", c=VW)[:, :, HD:HD + 1],
                ones_src[:].rearrange("p (h c) -> p h c", c=1),
            )

        # ---------- projection helpers ----------
        def load_x(sc):
            s0 = sc * 512
            x_t = xload.tile([128, N_DT, 512], FP16, tag="x")
            for dt in range(N_DT):
                nc.sync.dma_start(
                    out=x_t[:, dt, :],
                    in_=xT[dt * 128:(dt + 1) * 128, s0:s0 + 512],
                )
            return x_t

        def proj_qk(sc, x_t, name):
            s0 = sc * 512
            tp, ts = (qTp, qTs) if name == "q" else (kTp, kTs)
            for mt in range(2):
                ps_t = stp.tile([128, IC], F32, tag="st")
                ps = ps_t[:, 0:512]
                for dt in range(N_DT):
                    nc.tensor.matmul(
                        ps[:],
                        w_sb[name][:, dt, mt * 128:(mt + 1) * 128],
                        x_t[:, dt, :],
                        start=(dt == 0), stop=(dt == N_DT - 1),
                    )
                nc.scalar.copy(tp[:, mt, s0:s0 + 512], ps[:])
                nc.sync.dma_start(
                    out=ts[64:128, mt, s0:s0 + 512],
                    in_=tp[0:64, mt, s0:s0 + 512],
                )
                nc.sync.dma_start(
                    out=ts[0:64, mt, s0:s0 + 512],
                    in_=tp[64:128, mt, s0:s0 + 512],
                )

        def proj_v(sc, x_t):
            for st in range(4):
                ps_t = stp.tile([128, IC], F32, tag="st")
                ps = ps_t[:, 0:MPC]
                for dt in range(N_DT):
                    nc.tensor.matmul(
                        ps[:],
                        x_t[:, dt, st * 128:(st + 1) * 128],
                        w_sb["v"][:, dt, :],
                        start=(dt == 0), stop=(dt == N_DT - 1),
                    )
                jc = sc * 4 + st
                nc.vector.tensor_copy(
                    vaug[:, jc, :].rearrange(
                        "p (h c) -> p h c", c=VW)[:, :, 0:HD],
                    ps[:].rearrange("p (h d) -> p h d", d=HD),
                )

        # ---------- attention ----------
        # Software-pipelined: ST+exp of unit k are emitted, then the PV
        # of unit k-1 — so the in-order PE queue streams ST(k+1) while
        # exp(k) is still running instead of stalling in front of PV(k).
        exp_counter = [0]

        def attn_st_exp(h, ic, p):
            mt = h // 2
            even = (h % 2 == 0)
            k_lo = kTp if even else kTs
            k_hi = kTs if even else kTp
            q_lo = qTp if even else qTs
            q_hi = qTs if even else qTp
            jc0, jc1 = 2 * p, 2 * p + 1
            i0 = ic * IC
            st_hi = stp.tile([128, IC], F32, tag="st")
            st_lo = stp.tile([128, IC], F32, tag="st")
            for c0 in (0, 512):
                nc.tensor.matmul(
                    st_lo[:, c0:c0 + 512],
                    k_lo[0:64, mt, jc0 * 128:jc0 * 128 + 128],
                    q_lo[0:64, mt, i0 + c0:i0 + c0 + 512],
                    start=True, stop=True,
                )
                nc.tensor.matmul(
                    st_hi[:, c0:c0 + 512],
                    k_hi[64:128, mt, jc1 * 128:jc1 * 128 + 128],
                    q_hi[64:128, mt, i0 + c0:i0 + c0 + 512],
                    start=True, stop=True,
                )
            e_lo = esb.tile([128, IC], BF16, tag="e")
            e_hi = esb.tile([128, IC], BF16, tag="e")
            for st_t, e_t in ((st_lo, e_lo), (st_hi, e_hi)):
                g = exp_counter[0]
                exp_counter[0] += 1
                if (g * DVE_NUM) % DVE_DEN < DVE_NUM:
                    nc.vector.tensor_scalar(
                        e_t[:].bitcast(I16), st_t[:], EXP_A, EXP_B, MULT, ADD,
                    )
                else:
                    nc.scalar.activation(
                        e_t[:], st_t[:], Exp, bias=0.0, scale=SCALE)
            return (h, p, e_lo, e_hi)

        def attn_pv(pend, ot):
            h, p, e_lo, e_hi = pend
            jc0, jc1 = 2 * p, 2 * p + 1
            for jc, e_t in ((jc0, e_lo), (jc1, e_hi)):
                lhsT_v = vaug[:, jc, h * VW:h * VW + VW]
                for c0 in (0, 512):
                    nc.tensor.matmul(
                        ot[:, c0:c0 + 512],
                        lhsT_v,
                        e_t[:, c0:c0 + 512],
                        start=(jc == 0), stop=(jc == N_JC - 1),
                    )

        def epilogue(h, ic, ot):
            i0 = ic * IC
            ot_sb = episb.tile([VW, IC], BF16, tag="eo")
            nc.scalar.copy(ot_sb[:], ot[:])
            # transposes land in a bitcast view of an "st" slot so PSUM
            # stays within 8 banks; 256-col stride keeps blocks aligned.
            tr_t = stp.tile([128, IC], F32, tag="st")
            tr = tr_t[:].bitcast(BF16).rearrange("p (bi c) -> p bi c", c=256)
            for bi in range(IC // 128):
                nc.tensor.transpose(
                    tr[:, bi, 0:VW],
                    ot_sb[:, bi * 128:(bi + 1) * 128],
                    identb[0:VW, 0:VW],
                )
            rec = osb.tile([128, IC // 128], F32, tag="rec")
            nc.vector.reciprocal(rec[:], tr[:, :, HD])
            o_t = osb.tile([128, IC // 128, HD], F32, tag="o")
            nc.vector.tensor_tensor(
                o_t[:], tr[:, :, 0:HD],
                rec[:].unsqueeze(2).broadcast_to((128, IC // 128, HD)),
                MULT,
            )
            nc.sync.dma_start(
                out=out[i0:i0 + IC, h * HD:(h + 1) * HD].rearrange(
                    "(bi p) c -> p bi c", p=128),
                in_=o_t[:],
            )

        # ---------- schedule ----------
        # Single-pass projections (one x load per s-chunk feeds k, q and
        # v) woven into block (0,0)'s units; the PV stage runs one unit
        # behind ST/exp and is carried across block boundaries so the
        # in-order PE queue never waits on an exp it could prefetch past.
        state = {"pend": None, "prev_epi": None}

        def flush(ot):
            if state["pend"] is not None:
                attn_pv(state["pend"], ot)
                state["pend"] = None

        def run_block(h, ic, weave):
            ot = None
            for p in range(N_P):
                weave(p)
                cur = attn_st_exp(h, ic, p)
                if p == 0:
                    # finish the previous block (its last PV + epilogue)
                    # BEFORE allocating this block's ot from the 1-slot
                    # pool, so the slot-reuse dependency sees all readers
                    if state["prev_epi"] is not None:
                        ph, pic, pot = state["prev_epi"]
                        flush(pot)
                        epilogue(ph, pic, pot)
                        state["prev_epi"] = None
                    ot = otp.tile([VW, IC], F32, tag="ot")
                else:
                    flush(ot)
                state["pend"] = cur
            state["prev_epi"] = (h, ic, ot)

        # Dependency rule: the tile framework only creates dependencies
        # from a reader to writes EMITTED BEFORE it, so every projection
        # chunk must be emitted before its first consuming unit.
        # k/v chunk j covers units 2j..2j+1; q chunk j covers i-chunk j//2.
        def weave00(p):
            if p == 0:
                x0 = load_x(0)
                proj_qk(0, x0, "k")
                proj_qk(0, x0, "q")
                x1 = load_x(1)
                proj_qk(1, x1, "q")
                weave00.x = {0: x0, 1: x1}
            elif p == 1:
                proj_v(0, weave00.x[0])
            elif p % 2 == 0:
                j = p // 2
                if j + 1 <= 7:
                    xn = load_x(j + 1)
                    weave00.x[j + 1] = xn
                proj_qk(j, weave00.x[j], "k")
                proj_v(j, weave00.x[j])
            elif p >= 3:
                sc = (p + 1) // 2
                if sc <= 7:
                    proj_qk(sc, weave00.x[sc], "q")

        run_block(0, 0, weave00)
        for h in range(HPC):
            for ic in range(N_IC):
                if h == 0 and ic == 0:
                    continue
                run_block(h, ic, lambda p: None)
        # drain the pipeline
        h, ic, ot = state["prev_epi"]
        flush(ot)
        epilogue(h, ic, ot)


_NC_CACHE = None


def _get_nc():
    global _NC_CACHE
    if _NC_CACHE is None:
        _NC_CACHE = build_attention_kernel()
    return _NC_CACHE


def _build_in_maps(inputs):
    x = np.asarray(inputs["x"], dtype=np.float32)
    Wq = np.asarray(inputs["Wq"], dtype=np.float32)
    Wk = np.asarray(inputs["Wk"], dtype=np.float32)
    Wv = np.asarray(inputs["Wv"], dtype=np.float32)
    xTs = [np.ascontiguousarray(x[b].T).astype(np.float16)
           for b in range(N)]
    in_maps = []
    for c in range(N_CORES):
        b, g = divmod(c, N_CORES // N)
        rows = slice(g * MPC, (g + 1) * MPC)
        in_maps.append({
            "xT": xTs[b],
            "wqT": np.ascontiguousarray(Wq[rows].T).astype(np.float16),
            "wkT": np.ascontiguousarray(Wk[rows].T).astype(np.float16),
            "wvT": np.ascontiguousarray(Wv[rows].T).astype(np.float16),
        })
    return in_maps


def kernel(x, Wq, Wk, Wv):
    nc = _get_nc()
    in_maps = _build_in_maps({"x": x, "Wq": Wq, "Wk": Wk, "Wv": Wv})
    res = run_bass_kernel_spmd(nc, in_maps, core_ids=list(range(N_CORES)))

    full = np.empty((N, S, D), dtype=np.float32)
    for c in range(N_CORES):
        b, g = divmod(c, N_CORES // N)
        full[b, :, g * MPC:(g + 1) * MPC] = res.results[c]["out"]
    return full


if __name__ == "__main__":
    rng = np.random.default_rng(0)
    x = rng.standard_normal((N, S, D)).astype(np.float32)
    Wq = (rng.standard_normal((D, D)) / 32).astype(np.float32)
    Wk = (rng.standard_normal((D, D)) / 32).astype(np.float32)
    Wv = (rng.standard_normal((D, D)) / 32).astype(np.float32)
    got = kernel(x, Wq, Wk, Wv)
    print("kernel output:", got.shape, got.dtype)


# revision 20
# speedup vs baseline: 1.1734x; 1.1734x over previous
"""Multi-head self-attention (N=2, S=4096, D=1024, H=16) on 8 trn2 cores.

Sharding: data-parallel over batch (2) x tensor-parallel over head groups
(4 heads per core). Core c handles batch b=c//4, head group g=c%4
(heads 4g..4g+3, i.e. output columns 256g..256g+256). No cross-device
comms: heads are independent.

v3 design (vs the v2 "kTz zero-padded K=128" baseline):
  - ST score matmuls run at their true K=64 as row-tiled pairs: the jc
    and jc+1 key chunks of the SAME head execute concurrently in array
    rows 0-63 / 64-127 (tile_position (0,0)/(64,0)), halving ST time.
    This needs k^T and q^T present in BOTH partition halves: the
    projection writes the natural pair layout (even head lo / odd head
    hi) and a SBUF->SBUF DMA fills a swapped copy.
  - exp split across two engines: most chunks on ScalarE (ACTIVATE Exp,
    the accuracy reference), a tunable subset on VectorE via a
    Schraudolph bitcast exp: e = bitcast_bf16(int16(A*st + B)) in one
    tensor_scalar (mult, add) op. Per-element error ~3.3% max; applied
    to a fraction phi of key chunks the output rel-err grows as
    3.3%*sqrt(phi).
  - PV per head at M=65 (64 v columns + ones column accumulating the
    softmax denominator); single ot accumulator [65,1024].
  - PSUM: stp 3x[128,1024] (6 banks, also serves projection / epilogue
    scratch) + otp 1x[65,1024] (2 banks) = 8.
  - epilogue in bf16: PE-transpose [65,128] blocks, batched reciprocal
    over the 8 denominator columns, per-block scalar multiply, one DMA
    per (head, ic).
"""

import numpy as np

import concourse.bacc as bacc
import concourse.tile as tile
import concourse.mybir as mybir
from concourse.bass_utils import run_bass_kernel_spmd
from concourse.masks import make_identity

F32 = mybir.dt.float32
BF16 = mybir.dt.bfloat16
FP16 = mybir.dt.float16
I16 = mybir.dt.int16
Exp = mybir.ActivationFunctionType.Exp
MULT = mybir.AluOpType.mult
ADD = mybir.AluOpType.add

N, S, D = 2, 4096, 1024
H = 16
HD = D // H                      # 64
N_CORES = 8
HPC = H // (N_CORES // N)        # heads per core = 4
MPC = HPC * HD                   # out columns per core = 256
SCALE = 1.0 / np.sqrt(HD)        # post-matmul softmax scale

IC = 1024                        # i-chunk (query cols per exp instruction)
N_IC = S // IC                   # 4
N_JC = S // 128                  # 32 key chunks
N_P = N_JC // 2                  # 16 jc-pairs per (head, ic)
N_SC = S // 512                  # 8 projection s-chunks
N_DT = D // 128                  # 8 contraction tiles
VW = HD + 1                      # vaug stride per head (64 v + 1 ones)

# Schraudolph bitcast exp: e ~= bitcast_bf16(int16(EXP_A*st + EXP_B)).
EXP_A = float(SCALE * np.log2(np.e) * 128.0)
EXP_B = 16250.75
# every exp gets a global index g; it runs on the DVE when
# (g * DVE_NUM) % DVE_DEN < DVE_NUM evenly interleaves phi = NUM/DEN.
DVE_NUM, DVE_DEN = 1, 2          # phi = 0.5, strict alternation


def build_attention_kernel():
    nc = bacc.Bacc(
        "TRN2", target_bir_lowering=False, debug=False,
        enable_asserts=False, num_devices=N_CORES,
    )
    xT = nc.dram_tensor("xT", [D, S], FP16, kind="ExternalInput").ap()
    wqT = nc.dram_tensor("wqT", [D, MPC], FP16, kind="ExternalInput").ap()
    wkT = nc.dram_tensor("wkT", [D, MPC], FP16, kind="ExternalInput").ap()
    wvT = nc.dram_tensor("wvT", [D, MPC], FP16, kind="ExternalInput").ap()
    out = nc.dram_tensor("out", [S, MPC], F32, kind="ExternalOutput").ap()

    with tile.TileContext(nc) as tc:
        _emit(tc, xT, wqT, wkT, wvT, out)
    nc.compile()
    return nc


def _emit(tc, xT, wqT, wkT, wvT, out):
    nc = tc.nc
    with (
        tc.tile_pool(name="persist", bufs=1) as persist,
        # PSUM: stp 3x4KB slots (6 banks; ST tiles + projection ps +
        # epilogue transposes) + otp 1x[65,1024] (2 banks) = 8 banks.
        tc.tile_pool(name="stp", bufs=3, space="PSUM") as stp,
        tc.tile_pool(name="otp", bufs=1, space="PSUM") as otp,
        tc.tile_pool(name="xload", bufs=2) as xload,
        tc.tile_pool(name="esb", bufs=6) as esb,
        tc.tile_pool(name="episb", bufs=2) as episb,
        tc.tile_pool(name="osb", bufs=4) as osb,
    ):
        w_sb = {}
        for name, w in (("q", wqT), ("k", wkT), ("v", wvT)):
            t = persist.tile([128, N_DT, MPC], FP16, tag=f"w{name}")
            for dt in range(N_DT):
                nc.sync.dma_start(out=t[:, dt, :], in_=w[dt * 128:(dt + 1) * 128, :])
            w_sb[name] = t
        # pair layout: plane mt, partitions 0-63 = head 2mt, 64-127 =
        # head 2mt+1. swap layout: partition halves exchanged (via DMA).
        qTp = persist.tile([128, 2, S], FP16, tag="qTp")
        qTs = persist.tile([128, 2, S], FP16, tag="qTs")
        kTp = persist.tile([128, 2, S], FP16, tag="kTp")
        kTs = persist.tile([128, 2, S], FP16, tag="kTs")
        vaug = persist.tile([128, N_JC, HPC * VW], BF16, tag="vaug")
        ident = persist.tile([128, 128], F32, tag="ident")
        make_identity(nc, ident)
        identb = persist.tile([128, 128], BF16, tag="identb")
        nc.vector.tensor_copy(identb[:], ident[:])

        ones_src = persist.tile([128, HPC], F32, tag="ones")
        nc.vector.memset(ones_src, 1.0)
        for jc in range(N_JC):          # vaug ones columns
            nc.vector.tensor_copy(
                vaug[:, jc, :].rearrange(
                    "p (h c) -> p h c", c=VW)[:, :, HD:HD + 1],
                ones_src[:].rearrange("p (h c) -> p h c", c=1),
            )

        # ---------- projection helpers ----------
        def load_x(sc):
            s0 = sc * 512
            x_t = xload.tile([128, N_DT, 512], FP16, tag="x")
            for dt in range(N_DT):
                nc.sync.dma_start(
                    out=x_t[:, dt, :],
                    in_=xT[dt * 128:(dt + 1) * 128, s0:s0 + 512],
                )
            return x_t

        def proj_qk(sc, x_t, name):
            s0 = sc * 512
            tp, ts = (qTp, qTs) if name == "q" else (kTp, kTs)
            for mt in range(2):
                ps_t = stp.tile([128, IC], F32, tag="st")
                ps = ps_t[:, 0:512]
                for dt in range(N_DT):
                    nc.tensor.matmul(
                        ps[:],
                        w_sb[name][:, dt, mt * 128:(mt + 1) * 128],
                        x_t[:, dt, :],
                        start=(dt == 0), stop=(dt == N_DT - 1),
                    )
                nc.scalar.copy(tp[:, mt, s0:s0 + 512], ps[:])
                nc.sync.dma_start(
                    out=ts[64:128, mt, s0:s0 + 512],
                    in_=tp[0:64, mt, s0:s0 + 512],
                )
                nc.sync.dma_start(
                    out=ts[0:64, mt, s0:s0 + 512],
                    in_=tp[64:128, mt, s0:s0 + 512],
                )

        def proj_v(sc, x_t):
            for st in range(4):
                ps_t = stp.tile([128, IC], F32, tag="st")
                ps = ps_t[:, 0:MPC]
                for dt in range(N_DT):
                    nc.tensor.matmul(
                        ps[:],
                        x_t[:, dt, st * 128:(st + 1) * 128],
                        w_sb["v"][:, dt, :],
                        start=(dt == 0), stop=(dt == N_DT - 1),
                    )
                jc = sc * 4 + st
                nc.vector.tensor_copy(
                    vaug[:, jc, :].rearrange(
                        "p (h c) -> p h c", c=VW)[:, :, 0:HD],
                    ps[:].rearrange("p (h d) -> p h d", d=HD),
                )

        # ---------- attention ----------
        # Software-pipelined: ST+exp of unit k are emitted, then the PV
        # of unit k-1 — so the in-order PE queue streams ST(k+1) while
        # exp(k) is still running instead of stalling in front of PV(k).
        exp_counter = [0]

        def attn_st_exp(h, ic, p):
            mt = h // 2
            even = (h % 2 == 0)
            k_lo = kTp if even else kTs
            k_hi = kTs if even else kTp
            q_lo = qTp if even else qTs
            q_hi = qTs if even else qTp
            jc0, jc1 = 2 * p, 2 * p + 1
            i0 = ic * IC
            st_lo = stp.tile([128, IC], F32, tag="st")
            st_hi = stp.tile([128, IC], F32, tag="st")
            for c0 in (0, 512):
                nc.tensor.matmul(
                    st_lo[:, c0:c0 + 512],
                    k_lo[0:64, mt, jc0 * 128:jc0 * 128 + 128],
                    q_lo[0:64, mt, i0 + c0:i0 + c0 + 512],
                    start=True, stop=True,
                )
                nc.tensor.matmul(
                    st_hi[:, c0:c0 + 512],
                    k_hi[64:128, mt, jc1 * 128:jc1 * 128 + 128],
                    q_hi[64:128, mt, i0 + c0:i0 + c0 + 512],
                    start=True, stop=True,
                )
            e_lo = esb.tile([128, IC], BF16, tag="e")
            e_hi = esb.tile([128, IC], BF16, tag="e")
            for st_t, e_t in ((st_lo, e_lo), (st_hi, e_hi)):
                g = exp_counter[0]
                exp_counter[0] += 1
                if (g * DVE_NUM) % DVE_DEN < DVE_NUM:
                    nc.vector.tensor_scalar(
                        e_t[:].bitcast(I16), st_t[:], EXP_A, EXP_B, MULT, ADD,
                    )
                else:
                    nc.scalar.activation(
                        e_t[:], st_t[:], Exp, bias=0.0, scale=SCALE)
            return (h, p, e_lo, e_hi)

        def attn_pv(pend, ot):
            h, p, e_lo, e_hi = pend
            jc0, jc1 = 2 * p, 2 * p + 1
            for jc, e_t in ((jc0, e_lo), (jc1, e_hi)):
                lhsT_v = vaug[:, jc, h * VW:h * VW + VW]
                for c0 in (0, 512):
                    nc.tensor.matmul(
                        ot[:, c0:c0 + 512],
                        lhsT_v,
                        e_t[:, c0:c0 + 512],
                        start=(jc == 0), stop=(jc == N_JC - 1),
                    )

        def epilogue(h, ic, ot):
            i0 = ic * IC
            ot_sb = episb.tile([VW, IC], BF16, tag="eo")
            nc.scalar.copy(ot_sb[:], ot[:])
            # transposes land in a bitcast view of an "st" slot so PSUM
            # stays within 8 banks; 256-col stride keeps blocks aligned.
            tr_t = stp.tile([128, IC], F32, tag="st")
            tr = tr_t[:].bitcast(BF16).rearrange("p (bi c) -> p bi c", c=256)
            for bi in range(IC // 128):
                nc.tensor.transpose(
                    tr[:, bi, 0:VW],
                    ot_sb[:, bi * 128:(bi + 1) * 128],
                    identb[0:VW, 0:VW],
                )
            rec = osb.tile([128, IC // 128], F32, tag="rec")
            nc.vector.reciprocal(rec[:], tr[:, :, HD])
            o_t = osb.tile([128, IC // 128, HD], F32, tag="o")
            nc.vector.tensor_tensor(
                o_t[:], tr[:, :, 0:HD],
                rec[:].unsqueeze(2).broadcast_to((128, IC // 128, HD)),
                MULT,
            )
            nc.sync.dma_start(
                out=out[i0:i0 + IC, h * HD:(h + 1) * HD].rearrange(
                    "(bi p) c -> p bi c", p=128),
                in_=o_t[:],
            )

        # ---------- schedule ----------
        # Single-pass projections (one x load per s-chunk feeds k, q and
        # v) woven into block (0,0)'s units; the PV stage runs one unit
        # behind ST/exp and is carried across block boundaries so the
        # in-order PE queue never waits on an exp it could prefetch past.
        state = {"pend": None, "prev_epi": None}

        def flush(ot):
            if state["pend"] is not None:
                attn_pv(state["pend"], ot)
                state["pend"] = None

        def run_block(h, ic, weave):
            ot = None
            for p in range(N_P):
                weave(p)
                cur = attn_st_exp(h, ic, p)
                if p == 0:
                    # finish the previous block (its last PV + epilogue)
                    # BEFORE allocating this block's ot from the 1-slot
                    # pool, so the slot-reuse dependency sees all readers
                    if state["prev_epi"] is not None:
                        ph, pic, pot = state["prev_epi"]
                        flush(pot)
                        epilogue(ph, pic, pot)
                        state["prev_epi"] = None
                    ot = otp.tile([VW, IC], F32, tag="ot")
                else:
                    flush(ot)
                state["pend"] = cur
            state["prev_epi"] = (h, ic, ot)

        # Dependency rule: the tile framework only creates dependencies
        # from a reader to writes EMITTED BEFORE it, so every projection
        # chunk must be emitted before its first consuming unit.
        # k/v chunk j covers units 2j..2j+1; q chunk j covers i-chunk j//2.
        def weave00(p):
            if p == 0:
                x0 = load_x(0)
                proj_qk(0, x0, "k")
                proj_qk(0, x0, "q")
                x1 = load_x(1)
                proj_qk(1, x1, "k")
                proj_qk(1, x1, "q")
                proj_v(0, x0)
                weave00.x = {0: x0, 1: x1}
            elif p % 2 == 0:
                j = p // 2
                if j + 1 <= 7:
                    xn = load_x(j + 1)
                    proj_qk(j + 1, xn, "k")
                    weave00.x[j + 1] = xn
                proj_v(j, weave00.x[j])
            elif p >= 3:
                sc = (p + 1) // 2
                if sc <= 7:
                    proj_qk(sc, weave00.x[sc], "q")

        run_block(0, 0, weave00)
        for h in range(HPC):
            for ic in range(N_IC):
                if h == 0 and ic == 0:
                    continue
                run_block(h, ic, lambda p: None)
        # drain the pipeline
        h, ic, ot = state["prev_epi"]
        flush(ot)
        epilogue(h, ic, ot)


_NC_CACHE = None


def _get_nc():
    global _NC_CACHE
    if _NC_CACHE is None:
        _NC_CACHE = build_attention_kernel()
    return _NC_CACHE


def _build_in_maps(inputs):
    x = np.asarray(inputs["x"], dtype=np.float32)
    Wq = np.asarray(inputs["Wq"], dtype=np.float32)
    Wk = np.asarray(inputs["Wk"], dtype=np.float32)
    Wv = np.asarray(inputs["Wv"], dtype=np.float32)
    xTs = [np.ascontiguousarray(x[b].T).astype(np.float16)
           for b in range(N)]
    in_maps = []
    for c in range(N_CORES):
        b, g = divmod(c, N_CORES // N)
        rows = slice(g * MPC, (g + 1) * MPC)
        in_maps.append({
            "xT": xTs[b],
            "wqT": np.ascontiguousarray(Wq[rows].T).astype(np.float16),
            "wkT": np.ascontiguousarray(Wk[rows].T).astype(np.float16),
            "wvT": np.ascontiguousarray(Wv[rows].T).astype(np.float16),
        })
    return in_maps


def kernel(x, Wq, Wk, Wv):
    nc = _get_nc()
    in_maps = _build_in_maps({"x": x, "Wq": Wq, "Wk": Wk, "Wv": Wv})
    res = run_bass_kernel_spmd(nc, in_maps, core_ids=list(range(N_CORES)))

    full = np.empty((N, S, D), dtype=np.float32)
    for c in range(N_CORES):
        b, g = divmod(c, N_CORES // N)
        full[b, :, g * MPC:(g + 1) * MPC] = res.results[c]["out"]
    return full


if __name__ == "__main__":
    rng = np.random.default_rng(0)
    x = rng.standard_normal((N, S, D)).astype(np.float32)
    Wq = (rng.standard_normal((D, D)) / 32).astype(np.float32)
    Wk = (rng.standard_normal((D, D)) / 32).astype(np.float32)
    Wv = (rng.standard_normal((D, D)) / 32).astype(np.float32)
    got = kernel(x, Wq, Wk, Wv)
    print("kernel output:", got.shape, got.dtype)


# revision 21
# speedup vs baseline: 1.1918x; 1.0157x over previous
"""Multi-head self-attention (N=2, S=4096, D=1024, H=16) on 8 trn2 cores.

Sharding: data-parallel over batch (2) x tensor-parallel over head groups
(4 heads per core). Core c handles batch b=c//4, head group g=c%4
(heads 4g..4g+3, i.e. output columns 256g..256g+256). No cross-device
comms: heads are independent.

v3 design (vs the v2 "kTz zero-padded K=128" baseline):
  - ST score matmuls run at their true K=64 as row-tiled pairs: the jc
    and jc+1 key chunks of the SAME head execute concurrently in array
    rows 0-63 / 64-127 (tile_position (0,0)/(64,0)), halving ST time.
    This needs k^T and q^T present in BOTH partition halves: the
    projection writes the natural pair layout (even head lo / odd head
    hi) and a SBUF->SBUF DMA fills a swapped copy.
  - exp split across two engines: most chunks on ScalarE (ACTIVATE Exp,
    the accuracy reference), a tunable subset on VectorE via a
    Schraudolph bitcast exp: e = bitcast_bf16(int16(A*st + B)) in one
    tensor_scalar (mult, add) op. Per-element error ~3.3% max; applied
    to a fraction phi of key chunks the output rel-err grows as
    3.3%*sqrt(phi).
  - PV per head at M=65 (64 v columns + ones column accumulating the
    softmax denominator); single ot accumulator [65,1024].
  - PSUM: stp 3x[128,1024] (6 banks, also serves projection / epilogue
    scratch) + otp 1x[65,1024] (2 banks) = 8.
  - epilogue in bf16: PE-transpose [65,128] blocks, batched reciprocal
    over the 8 denominator columns, per-block scalar multiply, one DMA
    per (head, ic).
"""

import numpy as np

import concourse.bacc as bacc
import concourse.tile as tile
import concourse.mybir as mybir
from concourse.bass_utils import run_bass_kernel_spmd
from concourse.masks import make_identity

F32 = mybir.dt.float32
BF16 = mybir.dt.bfloat16
FP16 = mybir.dt.float16
I16 = mybir.dt.int16
Exp = mybir.ActivationFunctionType.Exp
MULT = mybir.AluOpType.mult
ADD = mybir.AluOpType.add

N, S, D = 2, 4096, 1024
H = 16
HD = D // H                      # 64
N_CORES = 8
HPC = H // (N_CORES // N)        # heads per core = 4
MPC = HPC * HD                   # out columns per core = 256
SCALE = 1.0 / np.sqrt(HD)        # post-matmul softmax scale

IC = 1024                        # i-chunk (query cols per exp instruction)
N_IC = S // IC                   # 4
N_JC = S // 128                  # 32 key chunks
N_P = N_JC // 2                  # 16 jc-pairs per (head, ic)
N_SC = S // 512                  # 8 projection s-chunks
N_DT = D // 128                  # 8 contraction tiles
VW = HD + 1                      # vaug stride per head (64 v + 1 ones)

# Schraudolph bitcast exp: e ~= bitcast_bf16(int16(EXP_A*st + EXP_B)).
EXP_A = float(SCALE * np.log2(np.e) * 128.0)
EXP_B = 16250.75
# every exp gets a global index g; it runs on the DVE when
# (g * DVE_NUM) % DVE_DEN < DVE_NUM evenly interleaves phi = NUM/DEN.
DVE_NUM, DVE_DEN = 1, 2          # phi = 0.5, strict alternation


def build_attention_kernel():
    nc = bacc.Bacc(
        "TRN2", target_bir_lowering=False, debug=False,
        enable_asserts=False, num_devices=N_CORES,
    )
    xT = nc.dram_tensor("xT", [D, S], FP16, kind="ExternalInput").ap()
    wqT = nc.dram_tensor("wqT", [D, MPC], FP16, kind="ExternalInput").ap()
    wkT = nc.dram_tensor("wkT", [D, MPC], FP16, kind="ExternalInput").ap()
    wvT = nc.dram_tensor("wvT", [D, MPC], FP16, kind="ExternalInput").ap()
    out = nc.dram_tensor("out", [S, MPC], F32, kind="ExternalOutput").ap()

    with tile.TileContext(nc) as tc:
        _emit(tc, xT, wqT, wkT, wvT, out)
    nc.compile()
    return nc


def _emit(tc, xT, wqT, wkT, wvT, out):
    nc = tc.nc
    with (
        tc.tile_pool(name="persist", bufs=1) as persist,
        # PSUM: stp 3x4KB slots (6 banks; ST tiles + projection ps +
        # epilogue transposes) + otp 1x[65,1024] (2 banks) = 8 banks.
        tc.tile_pool(name="stp", bufs=3, space="PSUM") as stp,
        tc.tile_pool(name="otp", bufs=1, space="PSUM") as otp,
        tc.tile_pool(name="xload", bufs=2) as xload,
        tc.tile_pool(name="esb", bufs=6) as esb,
        tc.tile_pool(name="episb", bufs=2) as episb,
        tc.tile_pool(name="osb", bufs=4) as osb,
    ):
        w_sb = {}
        for name, w in (("q", wqT), ("k", wkT), ("v", wvT)):
            t = persist.tile([128, N_DT, MPC], FP16, tag=f"w{name}")
            for dt in range(N_DT):
                nc.sync.dma_start(out=t[:, dt, :], in_=w[dt * 128:(dt + 1) * 128, :])
            w_sb[name] = t
        # pair layout: plane mt, partitions 0-63 = head 2mt, 64-127 =
        # head 2mt+1. swap layout: partition halves exchanged (via DMA).
        qTp = persist.tile([128, 2, S], FP16, tag="qTp")
        qTs = persist.tile([128, 2, S], FP16, tag="qTs")
        kTp = persist.tile([128, 2, S], FP16, tag="kTp")
        kTs = persist.tile([128, 2, S], FP16, tag="kTs")
        vaug = persist.tile([128, N_JC, HPC * VW], BF16, tag="vaug")
        ident = persist.tile([128, 128], F32, tag="ident")
        make_identity(nc, ident)
        identb = persist.tile([128, 128], BF16, tag="identb")
        nc.vector.tensor_copy(identb[:], ident[:])

        ones_src = persist.tile([128, HPC], F32, tag="ones")
        nc.vector.memset(ones_src, 1.0)
        for jc in range(N_JC):          # vaug ones columns
            nc.vector.tensor_copy(
                vaug[:, jc, :].rearrange(
                    "p (h c) -> p h c", c=VW)[:, :, HD:HD + 1],
                ones_src[:].rearrange("p (h c) -> p h c", c=1),
            )

        # ---------- projection helpers ----------
        def load_x(sc):
            s0 = sc * 512
            x_t = xload.tile([128, N_DT, 512], FP16, tag="x")
            for dt in range(N_DT):
                nc.sync.dma_start(
                    out=x_t[:, dt, :],
                    in_=xT[dt * 128:(dt + 1) * 128, s0:s0 + 512],
                )
            return x_t

        def proj_qk(sc, x_t, name):
            s0 = sc * 512
            tp, ts = (qTp, qTs) if name == "q" else (kTp, kTs)
            for mt in range(2):
                ps_t = stp.tile([128, IC], F32, tag="st")
                ps = ps_t[:, 0:512]
                for dt in range(N_DT):
                    nc.tensor.matmul(
                        ps[:],
                        w_sb[name][:, dt, mt * 128:(mt + 1) * 128],
                        x_t[:, dt, :],
                        start=(dt == 0), stop=(dt == N_DT - 1),
                    )
                nc.scalar.copy(tp[:, mt, s0:s0 + 512], ps[:])
                nc.sync.dma_start(
                    out=ts[64:128, mt, s0:s0 + 512],
                    in_=tp[0:64, mt, s0:s0 + 512],
                )
                nc.sync.dma_start(
                    out=ts[0:64, mt, s0:s0 + 512],
                    in_=tp[64:128, mt, s0:s0 + 512],
                )

        def proj_v(sc, x_t):
            for st in range(4):
                ps_t = stp.tile([128, IC], F32, tag="st")
                ps = ps_t[:, 0:MPC]
                for dt in range(N_DT):
                    nc.tensor.matmul(
                        ps[:],
                        x_t[:, dt, st * 128:(st + 1) * 128],
                        w_sb["v"][:, dt, :],
                        start=(dt == 0), stop=(dt == N_DT - 1),
                    )
                jc = sc * 4 + st
                nc.vector.tensor_copy(
                    vaug[:, jc, :].rearrange(
                        "p (h c) -> p h c", c=VW)[:, :, 0:HD],
                    ps[:].rearrange("p (h d) -> p h d", d=HD),
                )

        # ---------- attention ----------
        # Software-pipelined: ST+exp of unit k are emitted, then the PV
        # of unit k-1 — so the in-order PE queue streams ST(k+1) while
        # exp(k) is still running instead of stalling in front of PV(k).
        exp_counter = [0]

        def attn_st_exp(h, ic, p):
            mt = h // 2
            even = (h % 2 == 0)
            k_lo = kTp if even else kTs
            k_hi = kTs if even else kTp
            q_lo = qTp if even else qTs
            q_hi = qTs if even else qTp
            jc0, jc1 = 2 * p, 2 * p + 1
            i0 = ic * IC
            st_lo = stp.tile([128, IC], F32, tag="st")
            st_hi = stp.tile([128, IC], F32, tag="st")
            # lo halves first, then hi: degrades gracefully whichever
            # slot the ring releases later (hi overlaps lo's 2nd stream)
            for c0 in (0, 512):
                nc.tensor.matmul(
                    st_lo[:, c0:c0 + 512],
                    k_lo[0:64, mt, jc0 * 128:jc0 * 128 + 128],
                    q_lo[0:64, mt, i0 + c0:i0 + c0 + 512],
                    start=True, stop=True,
                )
            for c0 in (0, 512):
                nc.tensor.matmul(
                    st_hi[:, c0:c0 + 512],
                    k_hi[64:128, mt, jc1 * 128:jc1 * 128 + 128],
                    q_hi[64:128, mt, i0 + c0:i0 + c0 + 512],
                    start=True, stop=True,
                )
            e_lo = esb.tile([128, IC], BF16, tag="e")
            e_hi = esb.tile([128, IC], BF16, tag="e")
            for st_t, e_t in ((st_lo, e_lo), (st_hi, e_hi)):
                g = exp_counter[0]
                exp_counter[0] += 1
                if (g * DVE_NUM) % DVE_DEN < DVE_NUM:
                    nc.vector.tensor_scalar(
                        e_t[:].bitcast(I16), st_t[:], EXP_A, EXP_B, MULT, ADD,
                    )
                else:
                    nc.scalar.activation(
                        e_t[:], st_t[:], Exp, bias=0.0, scale=SCALE)
            return (h, p, e_lo, e_hi)

        def attn_pv(pend, ot):
            h, p, e_lo, e_hi = pend
            jc0, jc1 = 2 * p, 2 * p + 1
            for jc, e_t in ((jc0, e_lo), (jc1, e_hi)):
                lhsT_v = vaug[:, jc, h * VW:h * VW + VW]
                for c0 in (0, 512):
                    nc.tensor.matmul(
                        ot[:, c0:c0 + 512],
                        lhsT_v,
                        e_t[:, c0:c0 + 512],
                        start=(jc == 0), stop=(jc == N_JC - 1),
                    )

        def epilogue(h, ic, ot):
            i0 = ic * IC
            ot_sb = episb.tile([VW, IC], BF16, tag="eo")
            nc.scalar.copy(ot_sb[:], ot[:])
            # transposes land in a bitcast view of an "st" slot so PSUM
            # stays within 8 banks; 256-col stride keeps blocks aligned.
            tr_t = stp.tile([128, IC], F32, tag="st")
            tr = tr_t[:].bitcast(BF16).rearrange("p (bi c) -> p bi c", c=256)
            for bi in range(IC // 128):
                nc.tensor.transpose(
                    tr[:, bi, 0:VW],
                    ot_sb[:, bi * 128:(bi + 1) * 128],
                    identb[0:VW, 0:VW],
                )
            rec = osb.tile([128, IC // 128], F32, tag="rec")
            nc.vector.reciprocal(rec[:], tr[:, :, HD])
            o_t = osb.tile([128, IC // 128, HD], F32, tag="o")
            nc.vector.tensor_tensor(
                o_t[:], tr[:, :, 0:HD],
                rec[:].unsqueeze(2).broadcast_to((128, IC // 128, HD)),
                MULT,
            )
            nc.sync.dma_start(
                out=out[i0:i0 + IC, h * HD:(h + 1) * HD].rearrange(
                    "(bi p) c -> p bi c", p=128),
                in_=o_t[:],
            )

        # ---------- schedule ----------
        # Single-pass projections (one x load per s-chunk feeds k, q and
        # v) woven into block (0,0)'s units; the PV stage runs one unit
        # behind ST/exp and is carried across block boundaries so the
        # in-order PE queue never waits on an exp it could prefetch past.
        state = {"pend": None, "prev_epi": None}

        def flush(ot):
            if state["pend"] is not None:
                attn_pv(state["pend"], ot)
                state["pend"] = None

        def run_block(h, ic, weave):
            ot = None
            for p in range(N_P):
                weave(p)
                cur = attn_st_exp(h, ic, p)
                if p == 0:
                    # finish the previous block (its last PV + epilogue)
                    # BEFORE allocating this block's ot from the 1-slot
                    # pool, so the slot-reuse dependency sees all readers
                    if state["prev_epi"] is not None:
                        ph, pic, pot = state["prev_epi"]
                        flush(pot)
                        epilogue(ph, pic, pot)
                        state["prev_epi"] = None
                    ot = otp.tile([VW, IC], F32, tag="ot")
                else:
                    flush(ot)
                state["pend"] = cur
            state["prev_epi"] = (h, ic, ot)

        # Dependency rule: the tile framework only creates dependencies
        # from a reader to writes EMITTED BEFORE it, so every projection
        # chunk must be emitted before its first consuming unit.
        # k/v chunk j covers units 2j..2j+1; q chunk j covers i-chunk j//2.
        def weave00(p):
            if p == 0:
                x0 = load_x(0)
                proj_qk(0, x0, "k")
                proj_qk(0, x0, "q")
                x1 = load_x(1)
                proj_qk(1, x1, "k")
                proj_qk(1, x1, "q")
                proj_v(0, x0)
                weave00.x = {0: x0, 1: x1}
            elif p % 2 == 0:
                j = p // 2
                if j + 1 <= 7:
                    xn = load_x(j + 1)
                    proj_qk(j + 1, xn, "k")
                    weave00.x[j + 1] = xn
                proj_v(j, weave00.x[j])
            elif p >= 3:
                sc = (p + 1) // 2
                if sc <= 7:
                    proj_qk(sc, weave00.x[sc], "q")

        run_block(0, 0, weave00)
        for h in range(HPC):
            for ic in range(N_IC):
                if h == 0 and ic == 0:
                    continue
                run_block(h, ic, lambda p: None)
        # drain the pipeline
        h, ic, ot = state["prev_epi"]
        flush(ot)
        epilogue(h, ic, ot)


_NC_CACHE = None


def _get_nc():
    global _NC_CACHE
    if _NC_CACHE is None:
        _NC_CACHE = build_attention_kernel()
    return _NC_CACHE


def _build_in_maps(inputs):
    x = np.asarray(inputs["x"], dtype=np.float32)
    Wq = np.asarray(inputs["Wq"], dtype=np.float32)
    Wk = np.asarray(inputs["Wk"], dtype=np.float32)
    Wv = np.asarray(inputs["Wv"], dtype=np.float32)
    xTs = [np.ascontiguousarray(x[b].T).astype(np.float16)
           for b in range(N)]
    in_maps = []
    for c in range(N_CORES):
        b, g = divmod(c, N_CORES // N)
        rows = slice(g * MPC, (g + 1) * MPC)
        in_maps.append({
            "xT": xTs[b],
            "wqT": np.ascontiguousarray(Wq[rows].T).astype(np.float16),
            "wkT": np.ascontiguousarray(Wk[rows].T).astype(np.float16),
            "wvT": np.ascontiguousarray(Wv[rows].T).astype(np.float16),
        })
    return in_maps


def kernel(x, Wq, Wk, Wv):
    nc = _get_nc()
    in_maps = _build_in_maps({"x": x, "Wq": Wq, "Wk": Wk, "Wv": Wv})
    res = run_bass_kernel_spmd(nc, in_maps, core_ids=list(range(N_CORES)))

    full = np.empty((N, S, D), dtype=np.float32)
    for c in range(N_CORES):
        b, g = divmod(c, N_CORES // N)
        full[b, :, g * MPC:(g + 1) * MPC] = res.results[c]["out"]
    return full


if __name__ == "__main__":
    rng = np.random.default_rng(0)
    x = rng.standard_normal((N, S, D)).astype(np.float32)
    Wq = (rng.standard_normal((D, D)) / 32).astype(np.float32)
    Wk = (rng.standard_normal((D, D)) / 32).astype(np.float32)
    Wv = (rng.standard_normal((D, D)) / 32).astype(np.float32)
    got = kernel(x, Wq, Wk, Wv)
    print("kernel output:", got.shape, got.dtype)


# revision 23
# speedup vs baseline: 1.1923x; 1.0004x over previous
"""Multi-head self-attention (N=2, S=4096, D=1024, H=16) on 8 trn2 cores.

Sharding: data-parallel over batch (2) x tensor-parallel over head groups
(4 heads per core). Core c handles batch b=c//4, head group g=c%4
(heads 4g..4g+3, i.e. output columns 256g..256g+256). No cross-device
comms: heads are independent.

Final design (~585us HW, rel err 1.29e-2; prior baseline 737us):
  - ST score matmuls run at their true K=64 as row-tiled pairs: the jc
    and jc+1 key chunks of the SAME head execute concurrently in array
    rows 0-63 / 64-127 (tile_position (0,0)/(64,0)), halving ST time.
    This needs k^T and q^T present in BOTH partition halves: the
    projection writes the natural pair layout (even head lo / odd head
    hi) and a SBUF->SBUF DMA fills a swapped copy.
  - exp split across two engines: most chunks on ScalarE (ACTIVATE Exp,
    the accuracy reference), a tunable subset on VectorE via a
    Schraudolph bitcast exp: e = bitcast_bf16(int16(A*st + B)) in one
    tensor_scalar (mult, add) op. Per-element error ~3.3% max; applied
    to a fraction phi of key chunks the output rel-err grows as
    3.3%*sqrt(phi).
  - PV per head at M=65 (64 v columns + ones column accumulating the
    softmax denominator); single ot accumulator [65,1024].
  - PSUM: stp 3x[128,1024] (6 banks, also serves projection / epilogue
    scratch) + otp 1x[65,1024] (2 banks) = 8.
  - epilogue in bf16: PE-transpose [65,128] blocks, batched reciprocal
    over the 8 denominator columns, per-block scalar multiply, one DMA
    per (head, ic).
"""

import numpy as np

import concourse.bacc as bacc
import concourse.tile as tile
import concourse.mybir as mybir
from concourse.bass_utils import run_bass_kernel_spmd
from concourse.masks import make_identity

F32 = mybir.dt.float32
BF16 = mybir.dt.bfloat16
FP16 = mybir.dt.float16
I16 = mybir.dt.int16
Exp = mybir.ActivationFunctionType.Exp
MULT = mybir.AluOpType.mult
ADD = mybir.AluOpType.add

N, S, D = 2, 4096, 1024
H = 16
HD = D // H                      # 64
N_CORES = 8
HPC = H // (N_CORES // N)        # heads per core = 4
MPC = HPC * HD                   # out columns per core = 256
SCALE = 1.0 / np.sqrt(HD)        # post-matmul softmax scale

IC = 1024                        # i-chunk (query cols per exp instruction)
N_IC = S // IC                   # 4
N_JC = S // 128                  # 32 key chunks
N_P = N_JC // 2                  # 16 jc-pairs per (head, ic)
N_SC = S // 512                  # 8 projection s-chunks
N_DT = D // 128                  # 8 contraction tiles
VW = HD + 1                      # vaug stride per head (64 v + 1 ones)

# Schraudolph bitcast exp: e ~= bitcast_bf16(int16(EXP_A*st + EXP_B)).
EXP_A = float(SCALE * np.log2(np.e) * 128.0)
EXP_B = 16250.75
# every exp gets a global index g; it runs on the DVE when
# (g * DVE_NUM) % DVE_DEN < DVE_NUM evenly interleaves phi = NUM/DEN.
DVE_NUM, DVE_DEN = 1, 2          # phi = 0.5, strict alternation


def build_attention_kernel():
    nc = bacc.Bacc(
        "TRN2", target_bir_lowering=False, debug=False,
        enable_asserts=False, num_devices=N_CORES,
    )
    xT = nc.dram_tensor("xT", [D, S], FP16, kind="ExternalInput").ap()
    wqT = nc.dram_tensor("wqT", [D, MPC], FP16, kind="ExternalInput").ap()
    wkT = nc.dram_tensor("wkT", [D, MPC], FP16, kind="ExternalInput").ap()
    wvT = nc.dram_tensor("wvT", [D, MPC], FP16, kind="ExternalInput").ap()
    out = nc.dram_tensor("out", [S, MPC], F32, kind="ExternalOutput").ap()

    with tile.TileContext(nc) as tc:
        _emit(tc, xT, wqT, wkT, wvT, out)
    nc.compile()
    return nc


def _emit(tc, xT, wqT, wkT, wvT, out):
    nc = tc.nc
    with (
        tc.tile_pool(name="persist", bufs=1) as persist,
        # PSUM: stp 3x4KB slots (6 banks; ST tiles + projection ps +
        # epilogue transposes) + otp 1x[65,1024] (2 banks) = 8 banks.
        tc.tile_pool(name="stp", bufs=3, space="PSUM") as stp,
        tc.tile_pool(name="otp", bufs=1, space="PSUM") as otp,
        tc.tile_pool(name="xload", bufs=2) as xload,
        tc.tile_pool(name="esb", bufs=6) as esb,
        tc.tile_pool(name="episb", bufs=2) as episb,
        tc.tile_pool(name="osb", bufs=4) as osb,
    ):
        w_sb = {}
        for name, w in (("q", wqT), ("k", wkT), ("v", wvT)):
            t = persist.tile([128, N_DT, MPC], FP16, tag=f"w{name}")
            for dt in range(N_DT):
                nc.sync.dma_start(out=t[:, dt, :], in_=w[dt * 128:(dt + 1) * 128, :])
            w_sb[name] = t
        # pair layout: plane mt, partitions 0-63 = head 2mt, 64-127 =
        # head 2mt+1. swap layout: partition halves exchanged (via DMA).
        qTp = persist.tile([128, 2, S], FP16, tag="qTp")
        qTs = persist.tile([128, 2, S], FP16, tag="qTs")
        kTp = persist.tile([128, 2, S], FP16, tag="kTp")
        kTs = persist.tile([128, 2, S], FP16, tag="kTs")
        vaug = persist.tile([128, N_JC, HPC * VW], BF16, tag="vaug")
        ident = persist.tile([128, 128], F32, tag="ident")
        make_identity(nc, ident)
        identb = persist.tile([128, 128], BF16, tag="identb")
        nc.vector.tensor_copy(identb[:], ident[:])

        ones_src = persist.tile([128, HPC], F32, tag="ones")
        nc.vector.memset(ones_src, 1.0)
        for jc in range(N_JC):          # vaug ones columns
            nc.vector.tensor_copy(
                vaug[:, jc, :].rearrange(
                    "p (h c) -> p h c", c=VW)[:, :, HD:HD + 1],
                ones_src[:].rearrange("p (h c) -> p h c", c=1),
            )

        # ---------- projection helpers ----------
        def load_x(sc):
            s0 = sc * 512
            x_t = xload.tile([128, N_DT, 512], FP16, tag="x")
            for dt in range(N_DT):
                nc.sync.dma_start(
                    out=x_t[:, dt, :],
                    in_=xT[dt * 128:(dt + 1) * 128, s0:s0 + 512],
                )
            return x_t

        def proj_qk(sc, x_t, name):
            s0 = sc * 512
            tp, ts = (qTp, qTs) if name == "q" else (kTp, kTs)
            for mt in range(2):
                ps_t = stp.tile([128, IC], F32, tag="st")
                ps = ps_t[:, 0:512]
                for dt in range(N_DT):
                    nc.tensor.matmul(
                        ps[:],
                        w_sb[name][:, dt, mt * 128:(mt + 1) * 128],
                        x_t[:, dt, :],
                        start=(dt == 0), stop=(dt == N_DT - 1),
                    )
                nc.scalar.copy(tp[:, mt, s0:s0 + 512], ps[:])
                nc.sync.dma_start(
                    out=ts[64:128, mt, s0:s0 + 512],
                    in_=tp[0:64, mt, s0:s0 + 512],
                )
                nc.sync.dma_start(
                    out=ts[0:64, mt, s0:s0 + 512],
                    in_=tp[64:128, mt, s0:s0 + 512],
                )

        def proj_v(sc, x_t):
            for st in range(4):
                ps_t = stp.tile([128, IC], F32, tag="st")
                ps = ps_t[:, 0:MPC]
                for dt in range(N_DT):
                    nc.tensor.matmul(
                        ps[:],
                        x_t[:, dt, st * 128:(st + 1) * 128],
                        w_sb["v"][:, dt, :],
                        start=(dt == 0), stop=(dt == N_DT - 1),
                    )
                jc = sc * 4 + st
                nc.vector.tensor_copy(
                    vaug[:, jc, :].rearrange(
                        "p (h c) -> p h c", c=VW)[:, :, 0:HD],
                    ps[:].rearrange("p (h d) -> p h d", d=HD),
                )

        # ---------- attention ----------
        # Software-pipelined: ST+exp of unit k are emitted, then the PV
        # of unit k-1 — so the in-order PE queue streams ST(k+1) while
        # exp(k) is still running instead of stalling in front of PV(k).
        exp_counter = [0]

        def attn_st_exp(h, ic, p):
            mt = h // 2
            even = (h % 2 == 0)
            k_lo = kTp if even else kTs
            k_hi = kTs if even else kTp
            q_lo = qTp if even else qTs
            q_hi = qTs if even else qTp
            jc0, jc1 = 2 * p, 2 * p + 1
            i0 = ic * IC
            # hi allocated first: its ring slot is the older release, so
            # the hi MMs are always ready to pair with the lo MMs in the
            # scheduler's readiness ordering
            st_hi = stp.tile([128, IC], F32, tag="st")
            st_lo = stp.tile([128, IC], F32, tag="st")
            # lo halves first, then hi: degrades gracefully whichever
            # slot the ring releases later (hi overlaps lo's 2nd stream)
            for c0 in (0, 512):
                nc.tensor.matmul(
                    st_lo[:, c0:c0 + 512],
                    k_lo[0:64, mt, jc0 * 128:jc0 * 128 + 128],
                    q_lo[0:64, mt, i0 + c0:i0 + c0 + 512],
                    start=True, stop=True,
                )
            for c0 in (0, 512):
                nc.tensor.matmul(
                    st_hi[:, c0:c0 + 512],
                    k_hi[64:128, mt, jc1 * 128:jc1 * 128 + 128],
                    q_hi[64:128, mt, i0 + c0:i0 + c0 + 512],
                    start=True, stop=True,
                )
            e_lo = esb.tile([128, IC], BF16, tag="e")
            e_hi = esb.tile([128, IC], BF16, tag="e")
            for st_t, e_t in ((st_lo, e_lo), (st_hi, e_hi)):
                g = exp_counter[0]
                exp_counter[0] += 1
                if (g * DVE_NUM) % DVE_DEN < DVE_NUM:
                    nc.vector.tensor_scalar(
                        e_t[:].bitcast(I16), st_t[:], EXP_A, EXP_B, MULT, ADD,
                    )
                else:
                    nc.scalar.activation(
                        e_t[:], st_t[:], Exp, bias=0.0, scale=SCALE)
            return (h, p, e_lo, e_hi)

        def attn_pv(pend, ot):
            h, p, e_lo, e_hi = pend
            jc0, jc1 = 2 * p, 2 * p + 1
            for jc, e_t in ((jc0, e_lo), (jc1, e_hi)):
                lhsT_v = vaug[:, jc, h * VW:h * VW + VW]
                for c0 in (0, 512):
                    nc.tensor.matmul(
                        ot[:, c0:c0 + 512],
                        lhsT_v,
                        e_t[:, c0:c0 + 512],
                        start=(jc == 0), stop=(jc == N_JC - 1),
                    )

        def epilogue(h, ic, ot):
            i0 = ic * IC
            ot_sb = episb.tile([VW, IC], BF16, tag="eo")
            nc.scalar.copy(ot_sb[:], ot[:])
            # transposes land in a bitcast view of an "st" slot so PSUM
            # stays within 8 banks; 256-col stride keeps blocks aligned.
            tr_t = stp.tile([128, IC], F32, tag="st")
            tr = tr_t[:].bitcast(BF16).rearrange("p (bi c) -> p bi c", c=256)
            for bi in range(IC // 128):
                nc.tensor.transpose(
                    tr[:, bi, 0:VW],
                    ot_sb[:, bi * 128:(bi + 1) * 128],
                    identb[0:VW, 0:VW],
                )
            rec = osb.tile([128, IC // 128], F32, tag="rec")
            nc.vector.reciprocal(rec[:], tr[:, :, HD])
            o_t = osb.tile([128, IC // 128, HD], F32, tag="o")
            nc.vector.tensor_tensor(
                o_t[:], tr[:, :, 0:HD],
                rec[:].unsqueeze(2).broadcast_to((128, IC // 128, HD)),
                MULT,
            )
            nc.sync.dma_start(
                out=out[i0:i0 + IC, h * HD:(h + 1) * HD].rearrange(
                    "(bi p) c -> p bi c", p=128),
                in_=o_t[:],
            )

        # ---------- schedule ----------
        # Single-pass projections (one x load per s-chunk feeds k, q and
        # v) woven into block (0,0)'s units; the PV stage runs one unit
        # behind ST/exp and is carried across block boundaries so the
        # in-order PE queue never waits on an exp it could prefetch past.
        state = {"pend": None, "prev_epi": None}

        def flush(ot):
            if state["pend"] is not None:
                attn_pv(state["pend"], ot)
                state["pend"] = None

        def run_block(h, ic, weave):
            ot = None
            for p in range(N_P):
                weave(p)
                cur = attn_st_exp(h, ic, p)
                if p == 0:
                    # finish the previous block (its last PV + epilogue)
                    # BEFORE allocating this block's ot from the 1-slot
                    # pool, so the slot-reuse dependency sees all readers
                    if state["prev_epi"] is not None:
                        ph, pic, pot = state["prev_epi"]
                        flush(pot)
                        epilogue(ph, pic, pot)
                        state["prev_epi"] = None
                    ot = otp.tile([VW, IC], F32, tag="ot")
                else:
                    flush(ot)
                state["pend"] = cur
            state["prev_epi"] = (h, ic, ot)

        # Dependency rule: the tile framework only creates dependencies
        # from a reader to writes EMITTED BEFORE it, so every projection
        # chunk must be emitted before its first consuming unit.
        # k/v chunk j covers units 2j..2j+1; q chunk j covers i-chunk j//2.
        def weave00(p):
            if p == 0:
                x0 = load_x(0)
                proj_qk(0, x0, "k")
                proj_qk(0, x0, "q")
                x1 = load_x(1)
                proj_qk(1, x1, "k")
                proj_qk(1, x1, "q")
                proj_v(0, x0)
                weave00.x = {0: x0, 1: x1}
            elif p % 2 == 0:
                j = p // 2
                if j + 1 <= 7:
                    xn = load_x(j + 1)
                    proj_qk(j + 1, xn, "k")
                    weave00.x[j + 1] = xn
                proj_v(j, weave00.x[j])
            elif p >= 3:
                sc = (p + 1) // 2
                if sc <= 7:
                    proj_qk(sc, weave00.x[sc], "q")

        run_block(0, 0, weave00)
        for h in range(HPC):
            for ic in range(N_IC):
                if h == 0 and ic == 0:
                    continue
                run_block(h, ic, lambda p: None)
        # drain the pipeline
        h, ic, ot = state["prev_epi"]
        flush(ot)
        epilogue(h, ic, ot)


_NC_CACHE = None


def _get_nc():
    global _NC_CACHE
    if _NC_CACHE is None:
        _NC_CACHE = build_attention_kernel()
    return _NC_CACHE


def _build_in_maps(inputs):
    x = np.asarray(inputs["x"], dtype=np.float32)
    Wq = np.asarray(inputs["Wq"], dtype=np.float32)
    Wk = np.asarray(inputs["Wk"], dtype=np.float32)
    Wv = np.asarray(inputs["Wv"], dtype=np.float32)
    xTs = [np.ascontiguousarray(x[b].T).astype(np.float16)
           for b in range(N)]
    in_maps = []
    for c in range(N_CORES):
        b, g = divmod(c, N_CORES // N)
        rows = slice(g * MPC, (g + 1) * MPC)
        in_maps.append({
            "xT": xTs[b],
            "wqT": np.ascontiguousarray(Wq[rows].T).astype(np.float16),
            "wkT": np.ascontiguousarray(Wk[rows].T).astype(np.float16),
            "wvT": np.ascontiguousarray(Wv[rows].T).astype(np.float16),
        })
    return in_maps


def kernel(x, Wq, Wk, Wv):
    nc = _get_nc()
    in_maps = _build_in_maps({"x": x, "Wq": Wq, "Wk": Wk, "Wv": Wv})
    res = run_bass_kernel_spmd(nc, in_maps, core_ids=list(range(N_CORES)))

    full = np.empty((N, S, D), dtype=np.float32)
    for c in range(N_CORES):
        b, g = divmod(c, N_CORES // N)
        full[b, :, g * MPC:(g + 1) * MPC] = res.results[c]["out"]
    return full


if __name__ == "__main__":
    rng = np.random.default_rng(0)
    x = rng.standard_normal((N, S, D)).astype(np.float32)
    Wq = (rng.standard_normal((D, D)) / 32).astype(np.float32)
    Wk = (rng.standard_normal((D, D)) / 32).astype(np.float32)
    Wv = (rng.standard_normal((D, D)) / 32).astype(np.float32)
    got = kernel(x, Wq, Wk, Wv)
    print("kernel output:", got.shape, got.dtype)


# revision 24
# speedup vs baseline: 1.2105x; 1.0152x over previous
"""Multi-head self-attention (N=2, S=4096, D=1024, H=16) on 8 trn2 cores.

Sharding: data-parallel over batch (2) x tensor-parallel over head groups
(4 heads per core). Core c handles batch b=c//4, head group g=c%4
(heads 4g..4g+3, i.e. output columns 256g..256g+256). No cross-device
comms: heads are independent.

Final design (~585us HW, rel err 1.29e-2; prior baseline 737us):
  - ST score matmuls run at their true K=64 as row-tiled pairs: the jc
    and jc+1 key chunks of the SAME head execute concurrently in array
    rows 0-63 / 64-127 (tile_position (0,0)/(64,0)), halving ST time.
    This needs k^T and q^T present in BOTH partition halves: the
    projection writes the natural pair layout (even head lo / odd head
    hi) and a SBUF->SBUF DMA fills a swapped copy.
  - exp split across two engines: most chunks on ScalarE (ACTIVATE Exp,
    the accuracy reference), a tunable subset on VectorE via a
    Schraudolph bitcast exp: e = bitcast_bf16(int16(A*st + B)) in one
    tensor_scalar (mult, add) op. Per-element error ~3.3% max; applied
    to a fraction phi of key chunks the output rel-err grows as
    3.3%*sqrt(phi).
  - PV per head at M=65 (64 v columns + ones column accumulating the
    softmax denominator); single ot accumulator [65,1024].
  - PSUM: stp 3x[128,1024] (6 banks, also serves projection / epilogue
    scratch) + otp 1x[65,1024] (2 banks) = 8.
  - epilogue in bf16: PE-transpose [65,128] blocks, batched reciprocal
    over the 8 denominator columns, per-block scalar multiply, one DMA
    per (head, ic).
"""

import numpy as np

import concourse.bacc as bacc
import concourse.tile as tile
import concourse.mybir as mybir
from concourse.bass_utils import run_bass_kernel_spmd
from concourse.masks import make_identity

F32 = mybir.dt.float32
BF16 = mybir.dt.bfloat16
FP16 = mybir.dt.float16
I16 = mybir.dt.int16
Exp = mybir.ActivationFunctionType.Exp
MULT = mybir.AluOpType.mult
ADD = mybir.AluOpType.add

N, S, D = 2, 4096, 1024
H = 16
HD = D // H                      # 64
N_CORES = 8
HPC = H // (N_CORES // N)        # heads per core = 4
MPC = HPC * HD                   # out columns per core = 256
SCALE = 1.0 / np.sqrt(HD)        # post-matmul softmax scale

IC = 1024                        # i-chunk (query cols per exp instruction)
N_IC = S // IC                   # 4
N_JC = S // 128                  # 32 key chunks
N_P = N_JC // 2                  # 16 jc-pairs per (head, ic)
N_SC = S // 512                  # 8 projection s-chunks
N_DT = D // 128                  # 8 contraction tiles
VW = HD + 1                      # vaug stride per head (64 v + 1 ones)

# Schraudolph bitcast exp: e ~= bitcast_bf16(int16(EXP_A*st + EXP_B)).
EXP_A = float(SCALE * np.log2(np.e) * 128.0)
EXP_B = 16250.75
# every exp gets a global index g; it runs on the DVE when
# (g * DVE_NUM) % DVE_DEN < DVE_NUM evenly interleaves phi = NUM/DEN.
DVE_NUM, DVE_DEN = 1, 2          # phi = 0.5, strict alternation


def build_attention_kernel():
    nc = bacc.Bacc(
        "TRN2", target_bir_lowering=False, debug=False,
        enable_asserts=False, num_devices=N_CORES,
    )
    xT = nc.dram_tensor("xT", [D, S], FP16, kind="ExternalInput").ap()
    wqT = nc.dram_tensor("wqT", [D, MPC], FP16, kind="ExternalInput").ap()
    wkT = nc.dram_tensor("wkT", [D, MPC], FP16, kind="ExternalInput").ap()
    wvT = nc.dram_tensor("wvT", [D, MPC], FP16, kind="ExternalInput").ap()
    out = nc.dram_tensor("out", [S, MPC], F32, kind="ExternalOutput").ap()

    with tile.TileContext(nc) as tc:
        _emit(tc, xT, wqT, wkT, wvT, out)
    nc.compile()
    return nc


def _emit(tc, xT, wqT, wkT, wvT, out):
    nc = tc.nc
    with (
        tc.tile_pool(name="persist", bufs=1) as persist,
        # PSUM: stp 3x4KB slots (6 banks; ST tiles + projection ps +
        # epilogue transposes) + otp 1x[65,1024] (2 banks) = 8 banks.
        tc.tile_pool(name="stp", bufs=3, space="PSUM") as stp,
        tc.tile_pool(name="otp", bufs=1, space="PSUM") as otp,
        tc.tile_pool(name="xload", bufs=2) as xload,
        tc.tile_pool(name="esb", bufs=6) as esb,
        tc.tile_pool(name="episb", bufs=2) as episb,
        tc.tile_pool(name="osb", bufs=4) as osb,
    ):
        w_sb = {}
        for name, w in (("q", wqT), ("k", wkT), ("v", wvT)):
            t = persist.tile([128, N_DT, MPC], FP16, tag=f"w{name}")
            for dt in range(N_DT):
                nc.sync.dma_start(out=t[:, dt, :], in_=w[dt * 128:(dt + 1) * 128, :])
            w_sb[name] = t
        # pair layout: plane mt, partitions 0-63 = head 2mt, 64-127 =
        # head 2mt+1. swap layout: partition halves exchanged (via DMA).
        qTp = persist.tile([128, 2, S], FP16, tag="qTp")
        qTs = persist.tile([128, 2, S], FP16, tag="qTs")
        kTp = persist.tile([128, 2, S], FP16, tag="kTp")
        kTs = persist.tile([128, 2, S], FP16, tag="kTs")
        vaug = persist.tile([128, N_JC, HPC * VW], BF16, tag="vaug")
        ident = persist.tile([128, 128], F32, tag="ident")
        make_identity(nc, ident)
        identb = persist.tile([128, 128], BF16, tag="identb")
        nc.vector.tensor_copy(identb[:], ident[:])

        ones_src = persist.tile([128, HPC], F32, tag="ones")
        nc.vector.memset(ones_src, 1.0)
        for jc in range(N_JC):          # vaug ones columns
            nc.vector.tensor_copy(
                vaug[:, jc, :].rearrange(
                    "p (h c) -> p h c", c=VW)[:, :, HD:HD + 1],
                ones_src[:].rearrange("p (h c) -> p h c", c=1),
            )

        # ---------- projection helpers ----------
        def load_x(sc):
            s0 = sc * 512
            x_t = xload.tile([128, N_DT, 512], FP16, tag="x")
            for dt in range(N_DT):
                nc.sync.dma_start(
                    out=x_t[:, dt, :],
                    in_=xT[dt * 128:(dt + 1) * 128, s0:s0 + 512],
                )
            return x_t

        def proj_qk(sc, x_t, name):
            s0 = sc * 512
            tp, ts = (qTp, qTs) if name == "q" else (kTp, kTs)
            for mt in range(2):
                ps_t = stp.tile([128, IC], F32, tag="st")
                ps = ps_t[:, 0:512]
                for dt in range(N_DT):
                    nc.tensor.matmul(
                        ps[:],
                        w_sb[name][:, dt, mt * 128:(mt + 1) * 128],
                        x_t[:, dt, :],
                        start=(dt == 0), stop=(dt == N_DT - 1),
                    )
                nc.scalar.copy(tp[:, mt, s0:s0 + 512], ps[:])
                nc.sync.dma_start(
                    out=ts[64:128, mt, s0:s0 + 512],
                    in_=tp[0:64, mt, s0:s0 + 512],
                )
                nc.sync.dma_start(
                    out=ts[0:64, mt, s0:s0 + 512],
                    in_=tp[64:128, mt, s0:s0 + 512],
                )

        def proj_v(sc, x_t):
            for st in range(4):
                ps_t = stp.tile([128, IC], F32, tag="st")
                ps = ps_t[:, 0:MPC]
                for dt in range(N_DT):
                    nc.tensor.matmul(
                        ps[:],
                        x_t[:, dt, st * 128:(st + 1) * 128],
                        w_sb["v"][:, dt, :],
                        start=(dt == 0), stop=(dt == N_DT - 1),
                    )
                jc = sc * 4 + st
                nc.vector.tensor_copy(
                    vaug[:, jc, :].rearrange(
                        "p (h c) -> p h c", c=VW)[:, :, 0:HD],
                    ps[:].rearrange("p (h d) -> p h d", d=HD),
                )

        # ---------- attention ----------
        # Software-pipelined: ST+exp of unit k are emitted, then the PV
        # of unit k-1 — so the in-order PE queue streams ST(k+1) while
        # exp(k) is still running instead of stalling in front of PV(k).
        exp_counter = [0]

        def attn_st_exp(h, ic, p):
            mt = h // 2
            even = (h % 2 == 0)
            k_lo = kTp if even else kTs
            k_hi = kTs if even else kTp
            q_lo = qTp if even else qTs
            q_hi = qTs if even else qTp
            jc0, jc1 = 2 * p, 2 * p + 1
            i0 = ic * IC
            # hi allocated first: its ring slot is the older release, so
            # the hi MMs are always ready to pair with the lo MMs in the
            # scheduler's readiness ordering
            st_hi = stp.tile([128, IC], F32, tag="st")
            st_lo = stp.tile([128, IC], F32, tag="st")
            # hi emitted first within each half: row groups alternate in
            # the priority order, keeping the in-order LDWEIGHTS pipe
            # unblocked whichever st slot the ring releases later
            for c0 in (0, 512):
                nc.tensor.matmul(
                    st_hi[:, c0:c0 + 512],
                    k_hi[64:128, mt, jc1 * 128:jc1 * 128 + 128],
                    q_hi[64:128, mt, i0 + c0:i0 + c0 + 512],
                    start=True, stop=True,
                )
                nc.tensor.matmul(
                    st_lo[:, c0:c0 + 512],
                    k_lo[0:64, mt, jc0 * 128:jc0 * 128 + 128],
                    q_lo[0:64, mt, i0 + c0:i0 + c0 + 512],
                    start=True, stop=True,
                )
            e_lo = esb.tile([128, IC], BF16, tag="e")
            e_hi = esb.tile([128, IC], BF16, tag="e")
            for st_t, e_t in ((st_lo, e_lo), (st_hi, e_hi)):
                g = exp_counter[0]
                exp_counter[0] += 1
                if (g * DVE_NUM) % DVE_DEN < DVE_NUM:
                    nc.vector.tensor_scalar(
                        e_t[:].bitcast(I16), st_t[:], EXP_A, EXP_B, MULT, ADD,
                    )
                else:
                    nc.scalar.activation(
                        e_t[:], st_t[:], Exp, bias=0.0, scale=SCALE)
            return (h, p, e_lo, e_hi)

        def attn_pv(pend, ot):
            h, p, e_lo, e_hi = pend
            jc0, jc1 = 2 * p, 2 * p + 1
            for jc, e_t in ((jc0, e_lo), (jc1, e_hi)):
                lhsT_v = vaug[:, jc, h * VW:h * VW + VW]
                for c0 in (0, 512):
                    nc.tensor.matmul(
                        ot[:, c0:c0 + 512],
                        lhsT_v,
                        e_t[:, c0:c0 + 512],
                        start=(jc == 0), stop=(jc == N_JC - 1),
                    )

        def epilogue(h, ic, ot):
            i0 = ic * IC
            ot_sb = episb.tile([VW, IC], BF16, tag="eo")
            nc.scalar.copy(ot_sb[:], ot[:])
            # transposes land in a bitcast view of an "st" slot so PSUM
            # stays within 8 banks; 256-col stride keeps blocks aligned.
            tr_t = stp.tile([128, IC], F32, tag="st")
            tr = tr_t[:].bitcast(BF16).rearrange("p (bi c) -> p bi c", c=256)
            for bi in range(IC // 128):
                nc.tensor.transpose(
                    tr[:, bi, 0:VW],
                    ot_sb[:, bi * 128:(bi + 1) * 128],
                    identb[0:VW, 0:VW],
                )
            rec = osb.tile([128, IC // 128], F32, tag="rec")
            nc.vector.reciprocal(rec[:], tr[:, :, HD])
            o_t = osb.tile([128, IC // 128, HD], F32, tag="o")
            nc.vector.tensor_tensor(
                o_t[:], tr[:, :, 0:HD],
                rec[:].unsqueeze(2).broadcast_to((128, IC // 128, HD)),
                MULT,
            )
            nc.sync.dma_start(
                out=out[i0:i0 + IC, h * HD:(h + 1) * HD].rearrange(
                    "(bi p) c -> p bi c", p=128),
                in_=o_t[:],
            )

        # ---------- schedule ----------
        # Single-pass projections (one x load per s-chunk feeds k, q and
        # v) woven into block (0,0)'s units; the PV stage runs one unit
        # behind ST/exp and is carried across block boundaries so the
        # in-order PE queue never waits on an exp it could prefetch past.
        state = {"pend": None, "prev_epi": None}

        def flush(ot):
            if state["pend"] is not None:
                attn_pv(state["pend"], ot)
                state["pend"] = None

        def run_block(h, ic, weave):
            ot = None
            for p in range(N_P):
                weave(p)
                cur = attn_st_exp(h, ic, p)
                if p == 0:
                    # finish the previous block (its last PV + epilogue)
                    # BEFORE allocating this block's ot from the 1-slot
                    # pool, so the slot-reuse dependency sees all readers
                    if state["prev_epi"] is not None:
                        ph, pic, pot = state["prev_epi"]
                        flush(pot)
                        epilogue(ph, pic, pot)
                        state["prev_epi"] = None
                    ot = otp.tile([VW, IC], F32, tag="ot")
                else:
                    flush(ot)
                state["pend"] = cur
            state["prev_epi"] = (h, ic, ot)

        # Dependency rule: the tile framework only creates dependencies
        # from a reader to writes EMITTED BEFORE it, so every projection
        # chunk must be emitted before its first consuming unit.
        # k/v chunk j covers units 2j..2j+1; q chunk j covers i-chunk j//2.
        def weave00(p):
            if p == 0:
                x0 = load_x(0)
                proj_qk(0, x0, "k")
                proj_qk(0, x0, "q")
                x1 = load_x(1)
                proj_qk(1, x1, "k")
                proj_qk(1, x1, "q")
                proj_v(0, x0)
                weave00.x = {0: x0, 1: x1}
            elif p % 2 == 0:
                j = p // 2
                if j + 1 <= 7:
                    xn = load_x(j + 1)
                    proj_qk(j + 1, xn, "k")
                    weave00.x[j + 1] = xn
                proj_v(j, weave00.x[j])
            elif p >= 3:
                sc = (p + 1) // 2
                if sc <= 7:
                    proj_qk(sc, weave00.x[sc], "q")

        run_block(0, 0, weave00)
        for h in range(HPC):
            for ic in range(N_IC):
                if h == 0 and ic == 0:
                    continue
                run_block(h, ic, lambda p: None)
        # drain the pipeline
        h, ic, ot = state["prev_epi"]
        flush(ot)
        epilogue(h, ic, ot)


_NC_CACHE = None


def _get_nc():
    global _NC_CACHE
    if _NC_CACHE is None:
        _NC_CACHE = build_attention_kernel()
    return _NC_CACHE


def _build_in_maps(inputs):
    x = np.asarray(inputs["x"], dtype=np.float32)
    Wq = np.asarray(inputs["Wq"], dtype=np.float32)
    Wk = np.asarray(inputs["Wk"], dtype=np.float32)
    Wv = np.asarray(inputs["Wv"], dtype=np.float32)
    xTs = [np.ascontiguousarray(x[b].T).astype(np.float16)
           for b in range(N)]
    in_maps = []
    for c in range(N_CORES):
        b, g = divmod(c, N_CORES // N)
        rows = slice(g * MPC, (g + 1) * MPC)
        in_maps.append({
            "xT": xTs[b],
            "wqT": np.ascontiguousarray(Wq[rows].T).astype(np.float16),
            "wkT": np.ascontiguousarray(Wk[rows].T).astype(np.float16),
            "wvT": np.ascontiguousarray(Wv[rows].T).astype(np.float16),
        })
    return in_maps


def kernel(x, Wq, Wk, Wv):
    nc = _get_nc()
    in_maps = _build_in_maps({"x": x, "Wq": Wq, "Wk": Wk, "Wv": Wv})
    res = run_bass_kernel_spmd(nc, in_maps, core_ids=list(range(N_CORES)))

    full = np.empty((N, S, D), dtype=np.float32)
    for c in range(N_CORES):
        b, g = divmod(c, N_CORES // N)
        full[b, :, g * MPC:(g + 1) * MPC] = res.results[c]["out"]
    return full


if __name__ == "__main__":
    rng = np.random.default_rng(0)
    x = rng.standard_normal((N, S, D)).astype(np.float32)
    Wq = (rng.standard_normal((D, D)) / 32).astype(np.float32)
    Wk = (rng.standard_normal((D, D)) / 32).astype(np.float32)
    Wv = (rng.standard_normal((D, D)) / 32).astype(np.float32)
    got = kernel(x, Wq, Wk, Wv)
    print("kernel output:", got.shape, got.dtype)
